# revision 3
# baseline (speedup 1.0000x reference)
"""Trainium2 Bass kernel for a Tsit5 NeuralODE (MLP vector field) — v2.

Differences vs v1 (baseline):
  - The 512-row shard is split into TWO independent 256-column chains per
    core; their dependency graphs never touch, so the Tile list-scheduler
    overlaps chain A's matmuls with chain B's activations/DVE work. f32r
    moving operands keep full PE rate at N=256.
  - ALL layer biases are folded into the matmuls: L0 via a 65th K-row of
    ones carried inside the y/z state tiles; L1/L2/L3 via a K=1 bias-row
    matmul issued FIRST in each PSUM accumulation group (constants, so it
    never sits on the critical path). Activations are then bias-free and
    process both M-tiles in ONE instruction ([128, 2, 256]).
  - b3 folded into L3 makes ps3 == k_j exactly, so the first fold of every
    RK accumulator becomes acc_i = hA_i1*ps3_1 + y: the 6 accumulator
    setup ops per step per chain vanish.
  - y state is kept directly in f32r (storage is f32 bits; PE rounds) and
    DMA'd out via a bitcast view — no f32 twin copies.
  - RK folds that are off the critical path get de-prioritized so the
    critical z-chain always wins the DVE.
"""

import numpy as np

import concourse.bass as bass
import concourse.tile as tile
from concourse import bacc, mybir
from concourse.bass_utils import run_bass_kernel_spmd

# Tsit5 tableau (must match the reference)
A21 = 0.161
A31, A32 = -0.008480655492356989, 0.335480655492357
A41, A42, A43 = 2.8971530571054935, -6.359448489975075, 4.3622954328695815
A51, A52, A53, A54 = 5.325864828439257, -11.748883564062828, 7.4955393428898365, -0.09249506636175525
A61, A62, A63, A64, A65 = 5.86145544294642, -12.92096931784711, 8.159367898576159, -0.071584973281401, -0.028269050394068383
B1, B2, B3, B4, B5, B6 = 0.09646076681806523, 0.01, 0.4798896504144996, 1.379008574103742, -3.290069515436081, 2.324710524099774

AD = {2: {1: A21}, 3: {1: A31, 2: A32}, 4: {1: A41, 2: A42, 3: A43},
      5: {1: A51, 2: A52, 3: A53, 4: A54},
      6: {1: A61, 2: A62, 3: A63, 4: A64, 5: A65}}
BCOEF = {1: B1, 2: B2, 3: B3, 4: B4, 5: B5, 6: B6}

# pre-acc pairs (i, j) with j <= i-2, fixed flat order for the hA table
PAIRS = [(i, j) for j in range(1, 5) for i in range(j + 2, 7)]
PAIRQ = {p: q for q, p in enumerate(PAIRS)}

NCORES = 8
DIM, WIDTH = 64, 256
BATCH, NT = 4096, 101
NSTEP = NT - 1
SHARD = BATCH // NCORES      # 512 rows per core
NCH = 2                      # independent chains per core
CN = SHARD // NCH            # 256 columns per chain (f32r full rate at N>=256)

F32 = mybir.dt.float32
F32R = mybir.dt.float32r
MULT = mybir.AluOpType.mult
ADD = mybir.AluOpType.add
TANH = mybir.ActivationFunctionType.Tanh

FOLD_DEPRI = 4000

def _memset_ones(nc, ap):
    # neuronxcc rejects memset on f32r/bf16-typed APs; write the bit pattern
    if ap.dtype == F32R:
        nc.vector.memset(ap.bitcast(F32), 1.0)
    elif ap.dtype == mybir.dt.bfloat16:
        nc.vector.memset(ap.bitcast(mybir.dt.uint16), 0x3F80)
    else:
        nc.vector.memset(ap, 1.0)
  # priority penalty for off-critical-path RK folds

_cache = {}


def _build(nsteps=NSTEP, phase_ops=0, folds_pool=False):
    nc = bacc.Bacc("TRN2", target_bir_lowering=False, debug=False, num_devices=NCORES)

    y0t_d = nc.dram_tensor("y0t", [DIM, SHARD], F32, kind="ExternalInput").ap()
    hA_d = nc.dram_tensor("hA", [DIM, 21 * nsteps], F32, kind="ExternalInput").ap()
    w0b_d = nc.dram_tensor("w0b", [DIM + 1, WIDTH], F32, kind="ExternalInput").ap()
    w1_d = nc.dram_tensor("W1", [WIDTH, WIDTH], F32, kind="ExternalInput").ap()
    w2_d = nc.dram_tensor("W2", [WIDTH, WIDTH], F32, kind="ExternalInput").ap()
    w3_d = nc.dram_tensor("W3", [WIDTH, DIM], F32, kind="ExternalInput").ap()
    b1r_d = nc.dram_tensor("b1r", [1, WIDTH], F32, kind="ExternalInput").ap()
    b2r_d = nc.dram_tensor("b2r", [1, WIDTH], F32, kind="ExternalInput").ap()
    b3r_d = nc.dram_tensor("b3r", [1, DIM], F32, kind="ExternalInput").ap()
    out_d = nc.dram_tensor("ysT", [nsteps, DIM, SHARD], F32, kind="ExternalOutput").ap()

    with tile.TileContext(nc) as tc:
        with tc.tile_pool(name="const", bufs=1) as const, \
             tc.tile_pool(name="state", bufs=1) as state, \
             tc.tile_pool(name="psum", bufs=1, space="PSUM") as psum:

            # ---- load + round weights to f32r ----
            w0s = const.tile([DIM + 1, 2, 128], F32, tag="w0s")
            nc.sync.dma_start(w0s[:], w0b_d.rearrange("k (m j) -> k m j", j=128))
            w0 = const.tile([DIM + 1, 2, 128], F32R, tag="w0")
            nc.vector.tensor_copy(w0[:], w0s[:])

            w1 = const.tile([128, 2, 2, 128], F32R, tag="w1")
            w2 = const.tile([128, 2, 2, 128], F32R, tag="w2")
            for wd, wt, nm in ((w1_d, w1, "w1"), (w2_d, w2, "w2")):
                ws = const.tile([128, 2, 2, 128], F32, tag=nm + "s", name=nm + "s")
                for t in range(2):
                    nc.sync.dma_start(
                        ws[:, t],
                        wd[t * 128:(t + 1) * 128, :].rearrange("k (m j) -> k m j", j=128),
                    )
                nc.vector.tensor_copy(wt[:], ws[:])

            w3s = const.tile([128, 2, DIM], F32, tag="w3s")
            nc.sync.dma_start(w3s[:], w3_d.rearrange("(t k) d -> k t d", k=128))
            w3 = const.tile([128, 2, DIM], F32R, tag="w3")
            nc.vector.tensor_copy(w3[:], w3s[:])

            # bias rows (stationary K=1 operands)
            brow = {}
            for bd, nm, w in ((b1r_d, "b1r", 128), (b2r_d, "b2r", 128), (b3r_d, "b3r", DIM)):
                m = 2 if w == 128 else 1
                bs = const.tile([1, m, w], F32, tag=nm + "s", name=nm + "s")
                nc.sync.dma_start(bs[:], bd.rearrange("o (m j) -> o m j", j=w))
                br = const.tile([1, m, w], F32R, tag=nm, name=nm)
                nc.vector.tensor_copy(br[:], bs[:])
                brow[nm] = br

            # moving ones row for the K=1 bias matmuls
            ones = const.tile([1, CN], F32R, tag="ones")
            _memset_ones(nc, ones[:])

            # ---- per-step scalar table ----
            hA = const.tile([DIM, 21 * nsteps], F32, tag="hA")
            nc.sync.dma_start(hA[:], hA_d)

            # ---- per-chain state ----
            y0s = const.tile([DIM, SHARD], F32, tag="y0s")
            nc.sync.dma_start(y0s[:], y0t_d)

            y, z, acc, accy, h = [], [], [], [], []
            for c in range(NCH):
                yc = state.tile([DIM + 1, CN], F32R, tag=f"y{c}", name=f"y{c}")
                src_ap = y0s[:, c * CN:(c + 1) * CN]
                if c == 1 and phase_ops:
                    prev = src_ap
                    for d in range(phase_ops):
                        dt_ = state.tile([DIM, CN], F32, tag=f"dly{d}", name=f"dly{d}")
                        nc.vector.tensor_copy(dt_[:], prev)
                        prev = dt_[:]
                    src_ap = prev
                nc.vector.tensor_copy(yc[0:DIM], src_ap)
                _memset_ones(nc, yc[DIM:DIM + 1])
                y.append(yc)
                zc = state.tile([DIM + 1, CN], F32R, tag=f"z{c}", name=f"z{c}")
                _memset_ones(nc, zc[DIM:DIM + 1])
                z.append(zc)
                acc.append({
                    i: state.tile([DIM, CN], F32, tag=f"acc{i}_{c}", name=f"acc{i}_{c}")
                    for i in range(3, 7)
                })
                ac = state.tile([DIM, CN], F32, tag=f"accy{c}", name=f"accy{c}")
                accy.append(ac)
                h.append([
                    state.tile([128, 2, CN], F32R, tag=f"h{l}_{c}", name=f"h{l}_{c}")
                    for l in range(3)
                ])

            for t in range(nsteps):
                def sA(q):
                    return hA[:, q * nsteps + t: q * nsteps + t + 1]

                for s in range(1, 7):
                    for c in range(NCH):
                        rhs = y[c] if s == 1 else z[c]

                        # ---- MLP eval: biases inside the matmul groups ----
                        ps0 = psum.tile([128, 2, CN], F32, tag=f"psh{c}", name=f"ps0_{c}", bufs=2)
                        for m in range(2):
                            nc.tensor.matmul(ps0[:, m], w0[:, m], rhs[:], start=True, stop=True)
                        nc.scalar.activation(h[c][0][:], ps0[:], TANH)

                        ps1 = psum.tile([128, 2, CN], F32, tag=f"psh{c}", name=f"ps1_{c}", bufs=2)
                        for m in range(2):
                            nc.tensor.matmul(ps1[:, m], brow["b1r"][:, m], ones[:],
                                             start=True, stop=False)
                            for k in range(2):
                                nc.tensor.matmul(ps1[:, m], w1[:, k, m], h[c][0][:, k],
                                                 start=False, stop=(k == 1))
                        nc.scalar.activation(h[c][1][:], ps1[:], TANH)

                        ps2 = psum.tile([128, 2, CN], F32, tag=f"psh{c}", name=f"ps2_{c}", bufs=2)
                        for m in range(2):
                            nc.tensor.matmul(ps2[:, m], brow["b2r"][:, m], ones[:],
                                             start=True, stop=False)
                            for k in range(2):
                                nc.tensor.matmul(ps2[:, m], w2[:, k, m], h[c][1][:, k],
                                                 start=False, stop=(k == 1))
                        nc.scalar.activation(h[c][2][:], ps2[:], TANH)

                        ps3 = psum.tile([DIM, CN], F32, tag=f"ps3_{c}", name=f"ps3_{c}", bufs=2)
                        nc.tensor.matmul(ps3[:], brow["b3r"][:, 0], ones[:],
                                         start=True, stop=False)
                        for k in range(2):
                            nc.tensor.matmul(ps3[:], w3[:, k], h[c][2][:, k],
                                             start=False, stop=(k == 1))
                        # ps3 == k_s exactly (b3 included)

                        fold_eng = nc.gpsimd if folds_pool else nc.vector
                        # ---- fold k_s into the RK state ----
                        # first fold of each accumulator uses in1=y (no setup ops)
                        if s == 1:
                            nc.vector.scalar_tensor_tensor(
                                z[c][0:DIM], ps3[:], sA(0), y[c][0:DIM], MULT, ADD)
                            with tc.high_priority(offset=-FOLD_DEPRI):
                                for i in range(3, 7):
                                    fold_eng.scalar_tensor_tensor(
                                        acc[c][i][:], ps3[:], sA(5 + PAIRQ[(i, 1)]),
                                        y[c][0:DIM], MULT, ADD)
                                fold_eng.scalar_tensor_tensor(
                                    accy[c][:], ps3[:], sA(15), y[c][0:DIM], MULT, ADD)
                        elif s < 6:
                            nc.vector.scalar_tensor_tensor(
                                z[c][0:DIM], ps3[:], sA(s - 1), acc[c][s + 1][:], MULT, ADD)
                            with tc.high_priority(offset=-FOLD_DEPRI):
                                for i in range(s + 2, 7):
                                    fold_eng.scalar_tensor_tensor(
                                        acc[c][i][:], ps3[:], sA(5 + PAIRQ[(i, s)]),
                                        acc[c][i][:], MULT, ADD)
                                fold_eng.scalar_tensor_tensor(
                                    accy[c][:], ps3[:], sA(15 + s - 1), accy[c][:], MULT, ADD)
                        else:
                            nc.vector.scalar_tensor_tensor(
                                y[c][0:DIM], ps3[:], sA(20), accy[c][:], MULT, ADD)
                            nc.sync.dma_start(
                                out_d[t][:, c * CN:(c + 1) * CN],
                                y[c][0:DIM].bitcast(F32))

    nc.compile()
    return nc


def _get_nc(nsteps=NSTEP, **kw):
    key = (nsteps, tuple(sorted(kw.items())))
    if key not in _cache:
        _cache[key] = _build(nsteps, **kw)
    return _cache[key]


def _prepare_in_maps(ts, y0, W0, b0, W1, b1, W2, b2, W3, b3, nsteps=NSTEP):
    ts = np.asarray(ts, np.float32)
    hs = (ts[1:nsteps + 1] - ts[:nsteps]).astype(np.float64)          # [nsteps]
    # hA: [64, 21*nsteps]; q = 0..4: z-direct h*A_{i,i-1} (i=2..6);
    # q = 5..14: pre-acc h*A_ij per PAIRS; q = 15..19: h*B_j (j=1..5); q=20: h*B6
    cols = []
    for i in range(2, 7):
        cols.append(hs * AD[i][i - 1])
    for (i, j) in PAIRS:
        cols.append(hs * AD[i][j])
    for j in range(1, 6):
        cols.append(hs * BCOEF[j])
    cols.append(hs * B6)
    hA = np.concatenate([np.broadcast_to(c[None, :], (DIM, nsteps)) for c in cols],
                        axis=1).astype(np.float32)
    w0b = np.concatenate([np.asarray(W0, np.float32),
                          np.asarray(b0, np.float32)[None, :]], axis=0)
    common = {
        "hA": np.ascontiguousarray(hA),
        "w0b": np.ascontiguousarray(w0b),
        "W1": np.ascontiguousarray(W1, np.float32),
        "W2": np.ascontiguousarray(W2, np.float32),
        "W3": np.ascontiguousarray(W3, np.float32),
        "b1r": np.ascontiguousarray(np.asarray(b1, np.float32)[None, :]),
        "b2r": np.ascontiguousarray(np.asarray(b2, np.float32)[None, :]),
        "b3r": np.ascontiguousarray(np.asarray(b3, np.float32)[None, :]),
    }
    in_maps = []
    for i in range(NCORES):
        shard = np.asarray(y0[i * SHARD:(i + 1) * SHARD], np.float32)
        in_maps.append({"y0t": np.ascontiguousarray(shard.T), **common})
    return in_maps


def _run(inputs, nsteps=NSTEP, trace=False):
    nc = _get_nc(nsteps)
    in_maps = _prepare_in_maps(**inputs, nsteps=nsteps)
    res = run_bass_kernel_spmd(nc, in_maps, core_ids=list(range(NCORES)), trace=trace)
    y0 = np.asarray(inputs["y0"], np.float32)
    out = np.empty((nsteps + 1, BATCH, DIM), np.float32)
    out[0] = y0
    for i in range(NCORES):
        out[1:, i * SHARD:(i + 1) * SHARD, :] = res.results[i]["ysT"].transpose(0, 2, 1)
    return out, res


def kernel(**inputs) -> np.ndarray:
    out, _ = _run(inputs)
    return out


def _bench(inputs, iters=10, nsteps=NSTEP, **variant):
    """Time repeated device executes with a persistent jit + resident inputs.

    Returns (min_seconds_per_iter, all_times). Mirrors bass2jax.run_bass_via_pjrt's
    multi-core path but without donation so buffers stay resident across calls.
    """
    import jax
    import jax.numpy as jnp
    from jax.sharding import Mesh, PartitionSpec
    from jax.experimental.shard_map import shard_map
    from concourse import bass2jax
    from concourse import mybir as _mybir
    import time

    nc = _get_nc(nsteps, **variant)
    in_maps = _prepare_in_maps(**inputs, nsteps=nsteps)
    bass2jax.install_neuronx_cc_hook()

    partition_name = nc.partition_id_tensor.name if nc.partition_id_tensor else None
    in_names, out_names, out_avals = [], [], []
    for alloc in nc.m.functions[0].allocations:
        if not isinstance(alloc, _mybir.MemoryLocationSet):
            continue
        name = alloc.memorylocations[0].name
        if alloc.kind == "ExternalInput":
            if name != partition_name:
                in_names.append(name)
        elif alloc.kind == "ExternalOutput":
            out_names.append(name)
            out_avals.append(
                jax.core.ShapedArray(tuple(alloc.tensor_shape), _mybir.dt.np(alloc.dtype))
            )
    n_params = len(in_names)
    all_names = in_names + out_names
    if partition_name is not None:
        all_names = all_names + [partition_name]

    def _body(*args):
        operands = list(args)
        if partition_name is not None:
            operands.append(bass2jax.partition_id_tensor())
        return tuple(
            bass2jax._bass_exec_p.bind(
                *operands,
                out_avals=tuple(out_avals),
                in_names=tuple(all_names),
                out_names=tuple(out_names),
                lowering_input_output_aliases=(),
                sim_require_finite=True,
                sim_require_nnan=True,
                nc=nc,
            )
        )

    devices = jax.devices()[:NCORES]
    mesh = Mesh(np.asarray(devices), ("core",))
    n_outs = len(out_names)
    sharded = jax.jit(
        shard_map(
            _body,
            mesh=mesh,
            in_specs=(PartitionSpec("core"),) * (n_params + n_outs),
            out_specs=(PartitionSpec("core"),) * n_outs,
            check_rep=False,
        ),
        keep_unused=True,
    )
    concat_in = [
        jax.device_put(
            np.concatenate([np.asarray(in_maps[c][nm]) for c in range(NCORES)], axis=0)
        )
        for nm in in_names
    ]
    concat_zeros = [
        jax.device_put(np.zeros((NCORES * a.shape[0], *a.shape[1:]), a.dtype))
        for a in out_avals
    ]
    # warmup (compile)
    r = sharded(*concat_in, *concat_zeros)
    jax.block_until_ready(r)

    def run_n(n):
        t0 = time.perf_counter()
        rs = None
        for _ in range(n):
            rs = sharded(*concat_in, *concat_zeros)
        jax.block_until_ready(rs)
        return time.perf_counter() - t0

    run_n(3)  # pipeline warm
    slopes = []
    for _ in range(max(1, iters // 3)):
        t_small = run_n(5)
        t_big = run_n(25)
        slopes.append((t_big - t_small) / 20.0)
    return min(slopes), slopes



# revision 4
# speedup vs baseline: 2.8521x; 2.8521x over previous
"""Trainium2 Bass kernel for a Tsit5 NeuralODE (MLP vector field) — v2.

Differences vs v1 (baseline):
  - The 512-row shard is split into TWO independent 256-column chains per
    core; their dependency graphs never touch, so the Tile list-scheduler
    overlaps chain A's matmuls with chain B's activations/DVE work. f32r
    moving operands keep full PE rate at N=256.
  - ALL layer biases are folded into the matmuls: L0 via a 65th K-row of
    ones carried inside the y/z state tiles; L1/L2/L3 via a K=1 bias-row
    matmul issued FIRST in each PSUM accumulation group (constants, so it
    never sits on the critical path). Activations are then bias-free and
    process both M-tiles in ONE instruction ([128, 2, 256]).
  - b3 folded into L3 makes ps3 == k_j exactly, so the first fold of every
    RK accumulator becomes acc_i = hA_i1*ps3_1 + y: the 6 accumulator
    setup ops per step per chain vanish.
  - y state is kept directly in f32r (storage is f32 bits; PE rounds) and
    DMA'd out via a bitcast view — no f32 twin copies.
  - RK folds that are off the critical path get de-prioritized so the
    critical z-chain always wins the DVE.
"""

import numpy as np

import concourse.bass as bass
import concourse.tile as tile
from concourse import bacc, mybir
from concourse.bass_utils import run_bass_kernel_spmd

# Tsit5 tableau (must match the reference)
A21 = 0.161
A31, A32 = -0.008480655492356989, 0.335480655492357
A41, A42, A43 = 2.8971530571054935, -6.359448489975075, 4.3622954328695815
A51, A52, A53, A54 = 5.325864828439257, -11.748883564062828, 7.4955393428898365, -0.09249506636175525
A61, A62, A63, A64, A65 = 5.86145544294642, -12.92096931784711, 8.159367898576159, -0.071584973281401, -0.028269050394068383
B1, B2, B3, B4, B5, B6 = 0.09646076681806523, 0.01, 0.4798896504144996, 1.379008574103742, -3.290069515436081, 2.324710524099774

AD = {2: {1: A21}, 3: {1: A31, 2: A32}, 4: {1: A41, 2: A42, 3: A43},
      5: {1: A51, 2: A52, 3: A53, 4: A54},
      6: {1: A61, 2: A62, 3: A63, 4: A64, 5: A65}}
BCOEF = {1: B1, 2: B2, 3: B3, 4: B4, 5: B5, 6: B6}

# pre-acc pairs (i, j) with j <= i-2, fixed flat order for the hA table
PAIRS = [(i, j) for j in range(1, 5) for i in range(j + 2, 7)]
PAIRQ = {p: q for q, p in enumerate(PAIRS)}

NCORES = 8
DIM, WIDTH = 64, 256
BATCH, NT = 4096, 101
NSTEP = NT - 1
SHARD = BATCH // NCORES      # 512 rows per core
NCH = 2                      # independent chains per core
CN = SHARD // NCH            # 256 columns per chain (f32r full rate at N>=256)

F32 = mybir.dt.float32
F32R = mybir.dt.float32r
MULT = mybir.AluOpType.mult
ADD = mybir.AluOpType.add
TANH = mybir.ActivationFunctionType.Tanh

FOLD_DEPRI = 4000

def _memset_ones(nc, ap):
    # neuronxcc rejects memset on f32r/bf16-typed APs; write the bit pattern
    if ap.dtype == F32R:
        nc.vector.memset(ap.bitcast(F32), 1.0)
    elif ap.dtype == mybir.dt.bfloat16:
        nc.vector.memset(ap.bitcast(mybir.dt.uint16), 0x3F80)
    else:
        nc.vector.memset(ap, 1.0)
  # priority penalty for off-critical-path RK folds

_cache = {}


def _build(nsteps=NSTEP, phase_ops=0, folds_pool=False, outer_reps=1):
    nc = bacc.Bacc("TRN2", target_bir_lowering=False, debug=False, num_devices=NCORES)

    y0t_d = nc.dram_tensor("y0t", [DIM, SHARD], F32, kind="ExternalInput").ap()
    hA_d = nc.dram_tensor("hA", [DIM, 21 * nsteps], F32, kind="ExternalInput").ap()
    w0b_d = nc.dram_tensor("w0b", [DIM + 1, WIDTH], F32, kind="ExternalInput").ap()
    w1_d = nc.dram_tensor("W1", [WIDTH, WIDTH], F32, kind="ExternalInput").ap()
    w2_d = nc.dram_tensor("W2", [WIDTH, WIDTH], F32, kind="ExternalInput").ap()
    w3_d = nc.dram_tensor("W3", [WIDTH, DIM], F32, kind="ExternalInput").ap()
    b1r_d = nc.dram_tensor("b1r", [1, WIDTH], F32, kind="ExternalInput").ap()
    b2r_d = nc.dram_tensor("b2r", [1, WIDTH], F32, kind="ExternalInput").ap()
    b3r_d = nc.dram_tensor("b3r", [1, DIM], F32, kind="ExternalInput").ap()
    out_d = nc.dram_tensor("ysT", [nsteps, DIM, SHARD], F32, kind="ExternalOutput").ap()

    with tile.TileContext(nc) as tc:
        with tc.tile_pool(name="const", bufs=1) as const, \
             tc.tile_pool(name="state", bufs=1) as state, \
             tc.tile_pool(name="psum", bufs=1, space="PSUM") as psum:

            # ---- load + round weights to f32r ----
            w0s = const.tile([DIM + 1, 2, 128], F32, tag="w0s")
            nc.sync.dma_start(w0s[:], w0b_d.rearrange("k (m j) -> k m j", j=128))
            w0 = const.tile([DIM + 1, 2, 128], F32R, tag="w0")
            nc.vector.tensor_copy(w0[:], w0s[:])

            w1 = const.tile([128, 2, 2, 128], F32R, tag="w1")
            w2 = const.tile([128, 2, 2, 128], F32R, tag="w2")
            for wd, wt, nm in ((w1_d, w1, "w1"), (w2_d, w2, "w2")):
                ws = const.tile([128, 2, 2, 128], F32, tag=nm + "s", name=nm + "s")
                for t in range(2):
                    nc.sync.dma_start(
                        ws[:, t],
                        wd[t * 128:(t + 1) * 128, :].rearrange("k (m j) -> k m j", j=128),
                    )
                nc.vector.tensor_copy(wt[:], ws[:])

            w3s = const.tile([128, 2, DIM], F32, tag="w3s")
            nc.sync.dma_start(w3s[:], w3_d.rearrange("(t k) d -> k t d", k=128))
            w3 = const.tile([128, 2, DIM], F32R, tag="w3")
            nc.vector.tensor_copy(w3[:], w3s[:])

            # bias rows (stationary K=1 operands)
            brow = {}
            for bd, nm, w in ((b1r_d, "b1r", 128), (b2r_d, "b2r", 128), (b3r_d, "b3r", DIM)):
                m = 2 if w == 128 else 1
                bs = const.tile([1, m, w], F32, tag=nm + "s", name=nm + "s")
                nc.sync.dma_start(bs[:], bd.rearrange("o (m j) -> o m j", j=w))
                br = const.tile([1, m, w], F32R, tag=nm, name=nm)
                nc.vector.tensor_copy(br[:], bs[:])
                brow[nm] = br

            # moving ones row for the K=1 bias matmuls
            ones = const.tile([1, CN], F32R, tag="ones")
            _memset_ones(nc, ones[:])

            # ---- per-step scalar table ----
            hA = const.tile([DIM, 21 * nsteps], F32, tag="hA")
            nc.sync.dma_start(hA[:], hA_d)

            # ---- per-chain state ----
            y0s = const.tile([DIM, SHARD], F32, tag="y0s")
            nc.sync.dma_start(y0s[:], y0t_d)

            y, z, acc, accy, h = [], [], [], [], []
            for c in range(NCH):
                yc = state.tile([DIM + 1, CN], F32R, tag=f"y{c}", name=f"y{c}")
                src_ap = y0s[:, c * CN:(c + 1) * CN]
                if c == 1 and phase_ops:
                    prev = src_ap
                    for d in range(phase_ops):
                        dt_ = state.tile([DIM, CN], F32, tag=f"dly{d}", name=f"dly{d}")
                        nc.vector.tensor_copy(dt_[:], prev)
                        prev = dt_[:]
                    src_ap = prev
                nc.vector.tensor_copy(yc[0:DIM], src_ap)
                _memset_ones(nc, yc[DIM:DIM + 1])
                y.append(yc)
                zc = state.tile([DIM + 1, CN], F32R, tag=f"z{c}", name=f"z{c}")
                _memset_ones(nc, zc[DIM:DIM + 1])
                z.append(zc)
                acc.append({
                    i: state.tile([DIM, CN], F32, tag=f"acc{i}_{c}", name=f"acc{i}_{c}")
                    for i in range(3, 7)
                })
                ac = state.tile([DIM, CN], F32, tag=f"accy{c}", name=f"accy{c}")
                accy.append(ac)
                h.append([
                    state.tile([128, 2, CN], F32R, tag=f"h{l}_{c}", name=f"h{l}_{c}")
                    for l in range(3)
                ])

            import contextlib
            loop_cm = tc.For_i(0, outer_reps) if outer_reps > 1 else contextlib.nullcontext()
            with loop_cm:
              for t in range(nsteps):
                def sA(q):
                    return hA[:, q * nsteps + t: q * nsteps + t + 1]

                for s in range(1, 7):
                    for c in range(NCH):
                        rhs = y[c] if s == 1 else z[c]

                        # ---- MLP eval: biases inside the matmul groups ----
                        ps0 = psum.tile([128, 2, CN], F32, tag=f"psh{c}", name=f"ps0_{c}", bufs=2)
                        for m in range(2):
                            nc.tensor.matmul(ps0[:, m], w0[:, m], rhs[:], start=True, stop=True)
                        nc.scalar.activation(h[c][0][:], ps0[:], TANH)

                        ps1 = psum.tile([128, 2, CN], F32, tag=f"psh{c}", name=f"ps1_{c}", bufs=2)
                        for m in range(2):
                            nc.tensor.matmul(ps1[:, m], brow["b1r"][:, m], ones[:],
                                             start=True, stop=False)
                            for k in range(2):
                                nc.tensor.matmul(ps1[:, m], w1[:, k, m], h[c][0][:, k],
                                                 start=False, stop=(k == 1))
                        nc.scalar.activation(h[c][1][:], ps1[:], TANH)

                        ps2 = psum.tile([128, 2, CN], F32, tag=f"psh{c}", name=f"ps2_{c}", bufs=2)
                        for m in range(2):
                            nc.tensor.matmul(ps2[:, m], brow["b2r"][:, m], ones[:],
                                             start=True, stop=False)
                            for k in range(2):
                                nc.tensor.matmul(ps2[:, m], w2[:, k, m], h[c][1][:, k],
                                                 start=False, stop=(k == 1))
                        nc.scalar.activation(h[c][2][:], ps2[:], TANH)

                        ps3 = psum.tile([DIM, CN], F32, tag=f"ps3_{c}", name=f"ps3_{c}", bufs=2)
                        nc.tensor.matmul(ps3[:], brow["b3r"][:, 0], ones[:],
                                         start=True, stop=False)
                        for k in range(2):
                            nc.tensor.matmul(ps3[:], w3[:, k], h[c][2][:, k],
                                             start=False, stop=(k == 1))
                        # ps3 == k_s exactly (b3 included)

                        fold_eng = nc.gpsimd if folds_pool else nc.vector
                        # ---- fold k_s into the RK state ----
                        # first fold of each accumulator uses in1=y (no setup ops)
                        if s == 1:
                            nc.vector.scalar_tensor_tensor(
                                z[c][0:DIM], ps3[:], sA(0), y[c][0:DIM], MULT, ADD)
                            with tc.high_priority(offset=-FOLD_DEPRI):
                                for i in range(3, 7):
                                    fold_eng.scalar_tensor_tensor(
                                        acc[c][i][:], ps3[:], sA(5 + PAIRQ[(i, 1)]),
                                        y[c][0:DIM], MULT, ADD)
                                fold_eng.scalar_tensor_tensor(
                                    accy[c][:], ps3[:], sA(15), y[c][0:DIM], MULT, ADD)
                        elif s < 6:
                            nc.vector.scalar_tensor_tensor(
                                z[c][0:DIM], ps3[:], sA(s - 1), acc[c][s + 1][:], MULT, ADD)
                            with tc.high_priority(offset=-FOLD_DEPRI):
                                for i in range(s + 2, 7):
                                    fold_eng.scalar_tensor_tensor(
                                        acc[c][i][:], ps3[:], sA(5 + PAIRQ[(i, s)]),
                                        acc[c][i][:], MULT, ADD)
                                fold_eng.scalar_tensor_tensor(
                                    accy[c][:], ps3[:], sA(15 + s - 1), accy[c][:], MULT, ADD)
                        else:
                            nc.vector.scalar_tensor_tensor(
                                y[c][0:DIM], ps3[:], sA(20), accy[c][:], MULT, ADD)
                            nc.sync.dma_start(
                                out_d[t][:, c * CN:(c + 1) * CN],
                                y[c][0:DIM].bitcast(F32))

    nc.compile()
    return nc


def _get_nc(nsteps=NSTEP, **kw):
    key = (nsteps, tuple(sorted(kw.items())))
    if key not in _cache:
        _cache[key] = _build(nsteps, **kw)
    return _cache[key]


def _prepare_in_maps(ts, y0, W0, b0, W1, b1, W2, b2, W3, b3, nsteps=NSTEP):
    ts = np.asarray(ts, np.float32)
    hs = (ts[1:nsteps + 1] - ts[:nsteps]).astype(np.float64)          # [nsteps]
    # hA: [64, 21*nsteps]; q = 0..4: z-direct h*A_{i,i-1} (i=2..6);
    # q = 5..14: pre-acc h*A_ij per PAIRS; q = 15..19: h*B_j (j=1..5); q=20: h*B6
    cols = []
    for i in range(2, 7):
        cols.append(hs * AD[i][i - 1])
    for (i, j) in PAIRS:
        cols.append(hs * AD[i][j])
    for j in range(1, 6):
        cols.append(hs * BCOEF[j])
    cols.append(hs * B6)
    hA = np.concatenate([np.broadcast_to(c[None, :], (DIM, nsteps)) for c in cols],
                        axis=1).astype(np.float32)
    w0b = np.concatenate([np.asarray(W0, np.float32),
                          np.asarray(b0, np.float32)[None, :]], axis=0)
    common = {
        "hA": np.ascontiguousarray(hA),
        "w0b": np.ascontiguousarray(w0b),
        "W1": np.ascontiguousarray(W1, np.float32),
        "W2": np.ascontiguousarray(W2, np.float32),
        "W3": np.ascontiguousarray(W3, np.float32),
        "b1r": np.ascontiguousarray(np.asarray(b1, np.float32)[None, :]),
        "b2r": np.ascontiguousarray(np.asarray(b2, np.float32)[None, :]),
        "b3r": np.ascontiguousarray(np.asarray(b3, np.float32)[None, :]),
    }
    in_maps = []
    for i in range(NCORES):
        shard = np.asarray(y0[i * SHARD:(i + 1) * SHARD], np.float32)
        in_maps.append({"y0t": np.ascontiguousarray(shard.T), **common})
    return in_maps


def _run(inputs, nsteps=NSTEP, trace=False):
    nc = _get_nc(nsteps)
    in_maps = _prepare_in_maps(**inputs, nsteps=nsteps)
    res = run_bass_kernel_spmd(nc, in_maps, core_ids=list(range(NCORES)), trace=trace)
    y0 = np.asarray(inputs["y0"], np.float32)
    out = np.empty((nsteps + 1, BATCH, DIM), np.float32)
    out[0] = y0
    for i in range(NCORES):
        out[1:, i * SHARD:(i + 1) * SHARD, :] = res.results[i]["ysT"].transpose(0, 2, 1)
    return out, res


def kernel(**inputs) -> np.ndarray:
    out, _ = _run(inputs)
    return out


def _bench(inputs, iters=10, nsteps=NSTEP, **variant):
    """Time repeated device executes with a persistent jit + resident inputs.

    Returns (min_seconds_per_iter, all_times). Mirrors bass2jax.run_bass_via_pjrt's
    multi-core path but without donation so buffers stay resident across calls.
    """
    import jax
    import jax.numpy as jnp
    from jax.sharding import Mesh, PartitionSpec
    from jax.experimental.shard_map import shard_map
    from concourse import bass2jax
    from concourse import mybir as _mybir
    import time

    nc = _get_nc(nsteps, **variant)
    in_maps = _prepare_in_maps(**inputs, nsteps=nsteps)
    bass2jax.install_neuronx_cc_hook()

    partition_name = nc.partition_id_tensor.name if nc.partition_id_tensor else None
    in_names, out_names, out_avals = [], [], []
    for alloc in nc.m.functions[0].allocations:
        if not isinstance(alloc, _mybir.MemoryLocationSet):
            continue
        name = alloc.memorylocations[0].name
        if alloc.kind == "ExternalInput":
            if name != partition_name:
                in_names.append(name)
        elif alloc.kind == "ExternalOutput":
            out_names.append(name)
            out_avals.append(
                jax.core.ShapedArray(tuple(alloc.tensor_shape), _mybir.dt.np(alloc.dtype))
            )
    n_params = len(in_names)
    all_names = in_names + out_names
    if partition_name is not None:
        all_names = all_names + [partition_name]

    def _body(*args):
        operands = list(args)
        if partition_name is not None:
            operands.append(bass2jax.partition_id_tensor())
        return tuple(
            bass2jax._bass_exec_p.bind(
                *operands,
                out_avals=tuple(out_avals),
                in_names=tuple(all_names),
                out_names=tuple(out_names),
                lowering_input_output_aliases=(),
                sim_require_finite=True,
                sim_require_nnan=True,
                nc=nc,
            )
        )

    devices = jax.devices()[:NCORES]
    mesh = Mesh(np.asarray(devices), ("core",))
    n_outs = len(out_names)
    sharded = jax.jit(
        shard_map(
            _body,
            mesh=mesh,
            in_specs=(PartitionSpec("core"),) * (n_params + n_outs),
            out_specs=(PartitionSpec("core"),) * n_outs,
            check_rep=False,
        ),
        keep_unused=True,
    )
    concat_in = [
        jax.device_put(
            np.concatenate([np.asarray(in_maps[c][nm]) for c in range(NCORES)], axis=0)
        )
        for nm in in_names
    ]
    concat_zeros = [
        jax.device_put(np.zeros((NCORES * a.shape[0], *a.shape[1:]), a.dtype))
        for a in out_avals
    ]
    # warmup (compile)
    r = sharded(*concat_in, *concat_zeros)
    jax.block_until_ready(r)

    def run_n(n):
        t0 = time.perf_counter()
        rs = None
        for _ in range(n):
            rs = sharded(*concat_in, *concat_zeros)
        jax.block_until_ready(rs)
        return time.perf_counter() - t0

    run_n(3)  # pipeline warm
    slopes = []
    for _ in range(max(1, iters // 3)):
        t_small = run_n(5)
        t_big = run_n(25)
        slopes.append((t_big - t_small) / 20.0)
    return min(slopes), slopes

